# revision 1
# baseline (speedup 1.0000x reference)
"""Expected Calibration Error kernel for Trainium2 (Bass/Tile), 8 NeuronCores.

Problem: logits [1000000, 100] f32, labels [1000000] i64 ->
  (ece [1] f32, acc [1] f32)   (matching the jax reference's return tuple)

Strategy (data-parallel over rows):
  - Each core processes ROWS_CORE = 128*16*62 = 126976 rows; core 7's slice is
    padded with -1.0 logit rows (row-max = -1 fails every `conf > bound`
    comparison, so pad rows contribute exactly nothing to any bin).
  - Host precomputes chosen[i] = logits[i, labels[i]] (4 MB side input).
    On device, accuracy per row is (chosen == rowmax), which equals
    (argmax == label) whenever the row max is unique (verified for these
    deterministic inputs).
  - Device, per [128, 16, 100] tile: grouped reduce_max -> conf [128,16];
    acc = is_equal(chosen, conf); G = is_gt(conf, bounds) [128,16,16]
    (cumulative bin masks against the 16 linspace boundaries). TensorE
    accumulates G^T @ [conf, acc] and G^T @ ones into PSUM across all tiles.
  - Host folds the tiny [128,17] per-core outputs, differences the cumulative
    sums into the 15 bins, and applies the ECE formula.
"""

import numpy as np

P = 128          # SBUF partitions
C = 100          # classes
R = 16           # rows per partition per tile
T = 62           # tiles per core
NCORES = 8
NB = 16          # bin boundaries (15 bins)
ROWS_CORE = P * R * T          # 126976
N = 1_000_000
HALF = 8         # groups per matmul chunk (lhsT free dim = HALF*NB = 128)

_CACHE = {}


def _build_nc(reps=1, xbufs=4, do_vec=True, do_pe=True, dma_mode="sp"):
    import concourse.bass as bass
    import concourse.bacc as bacc
    import concourse.mybir as mybir
    import concourse.tile as tile

    f32 = mybir.dt.float32
    # Bacc (not plain Bass): its finalize() runs generate_event_semaphores,
    # which splits multi-wait sync onto event semaphores — walrus core_v3
    # codegen allows at most one sync wait per instruction.
    nc = bacc.Bacc()

    logits_d = nc.dram_tensor("logits", [ROWS_CORE, C], f32, kind="ExternalInput")
    chosen_d = nc.dram_tensor("chosen", [P, T * R], f32, kind="ExternalInput")
    bounds_d = nc.dram_tensor("bounds", [1, NB], f32, kind="ExternalInput")
    out_d = nc.dram_tensor("out", [P, NB + 1], f32, kind="ExternalOutput")

    # [T, 128, R*C] view of the row-major logits: partition p of tile t holds
    # rows t*(128*R) + p*R ... + R.
    lx = logits_d[:].flatten().rearrange("(t p f) -> t p f", t=T, p=P, f=R * C)

    with tile.TileContext(nc) as tc:
        with (
            tc.tile_pool(name="singles", bufs=1) as singles,
            tc.tile_pool(name="xtiles", bufs=xbufs) as xtiles,
            tc.tile_pool(name="vals", bufs=T) as valsp,
            tc.tile_pool(name="gmask", bufs=4) as gmaskp,
            tc.tile_pool(name="psum", bufs=1, space="PSUM") as psump,
        ):
            bounds_sb = singles.tile([P, NB], f32)
            nc.sync.dma_start(
                out=bounds_sb[:],
                in_=bass.AP(
                    tensor=bounds_d, offset=0, ap=[[0, P], [1, NB]]
                ),
            )
            chosen_sb = singles.tile([P, T * R], f32)
            nc.sync.dma_start(out=chosen_sb[:], in_=chosen_d[:])
            ones_sb = singles.tile([P, 1], f32)
            nc.vector.memset(ones_sb[:], 1.0)
            # First-touch of chosen_sb on DVE: carries the DMA-complete wait so
            # the in-loop is_equal ops never need a second sync-wait slot
            # (walrus core_v3 TensorTensor rejects 2 waits on one instruction).
            touch = singles.tile([P, 1], f32)
            nc.vector.tensor_copy(out=touch[:], in_=chosen_sb[:, 0:1])

            psum_stats = psump.tile([P, HALF * 2], f32)   # [128, 16]
            psum_cnt = psump.tile([P, 1], f32)

            bounds_b = bounds_sb[:].unsqueeze(1).broadcast_to([P, R, NB])

            for rep in range(reps):
              for t in range(T):
                x = xtiles.tile([P, R, C], f32)
                src = lx[t].rearrange("p (r c) -> p r c", r=R)
                if dma_mode == "sp":
                    nc.sync.dma_start(out=x[:], in_=src)
                elif dma_mode == "alt":
                    eng = nc.sync if t % 2 == 0 else nc.scalar
                    eng.dma_start(out=x[:], in_=src)
                elif dma_mode == "gpsimd":
                    nc.gpsimd.dma_start(out=x[:], in_=src)
                elif dma_mode == "split":
                    half = R // 2
                    nc.sync.dma_start(out=x[:, :half, :], in_=src[:, :half, :])
                    nc.scalar.dma_start(out=x[:, half:, :], in_=src[:, half:, :])
                else:
                    raise ValueError(dma_mode)

                if not do_vec:
                    # DMA-rate probe: consume one element per tile so the DMA
                    # completion is on the critical path (un-consumed DMA
                    # floods wedge the device).
                    dummy = gmaskp.tile([P, 1], f32)
                    nc.vector.tensor_copy(out=dummy[:], in_=x[:, 0:1, 0])
                    continue
                v = valsp.tile([P, R, 2], f32)
                conf = v[:, :, 0]
                nc.vector.tensor_reduce(
                    out=conf, in_=x[:], axis=mybir.AxisListType.X,
                    op=mybir.AluOpType.max,
                )
                nc.vector.tensor_tensor(
                    out=v[:, :, 1],
                    in0=chosen_sb[:, t * R:(t + 1) * R],
                    in1=conf,
                    op=mybir.AluOpType.is_equal,
                )
                g = gmaskp.tile([P, R, NB], f32)
                nc.vector.tensor_tensor(
                    out=g[:],
                    in0=conf.unsqueeze(2).broadcast_to([P, R, NB]),
                    in1=bounds_b,
                    op=mybir.AluOpType.is_gt,
                )
                nchunk = R // HALF
                for h in range(nchunk if do_pe else 0):
                    gh = g[:, h * HALF:(h + 1) * HALF, :].rearrange("p a b -> p (a b)")
                    vh = v[:, h * HALF:(h + 1) * HALF, :].rearrange("p a b -> p (a b)")
                    first = (rep == 0 and t == 0 and h == 0)
                    last = (rep == reps - 1 and t == T - 1 and h == nchunk - 1)
                    nc.tensor.matmul(
                        psum_stats[:], gh, vh, start=first, stop=last,
                    )
                    nc.tensor.matmul(
                        psum_cnt[:], gh, ones_sb[:], start=first, stop=last,
                    )

            out_sb = singles.tile([P, NB + 1], f32)
            if do_pe:
                nc.vector.tensor_copy(out=out_sb[:, 0:HALF * 2], in_=psum_stats[:])
                nc.vector.tensor_copy(out=out_sb[:, HALF * 2:HALF * 2 + 1], in_=psum_cnt[:])
            else:
                nc.vector.memset(out_sb[:], 0.0)
            nc.sync.dma_start(out=out_d[:], in_=out_sb[:])

    nc.finalize()
    return nc


def _get_nc():
    if "nc" not in _CACHE:
        _CACHE["nc"] = _build_nc()
    return _CACHE["nc"]


def _prep_inputs(logits, labels):
    """Shard + host-side prep. Returns in_maps for run_bass_kernel_spmd."""
    logits = np.asarray(logits)
    labels = np.asarray(labels)
    assert logits.shape == (N, C) and logits.dtype == np.float32

    bounds = np.linspace(0.0, 1.0, NB, dtype=np.float32)
    chosen = np.take_along_axis(
        logits, labels.reshape(-1, 1).astype(np.int64), axis=1
    ).reshape(-1)

    in_maps = []
    for c in range(NCORES):
        lo = c * ROWS_CORE
        hi = lo + ROWS_CORE
        if hi <= N:
            lg = logits[lo:hi]           # view, no copy
            ch = chosen[lo:hi]
        else:
            npad = hi - N
            lg = np.vstack([logits[lo:N], np.full((npad, C), -1.0, np.float32)])
            ch = np.concatenate([chosen[lo:N], np.zeros(npad, np.float32)])
        ch_t = np.ascontiguousarray(
            ch.reshape(T, P, R).transpose(1, 0, 2).reshape(P, T * R)
        )
        in_maps.append({"logits": lg, "chosen": ch_t, "bounds": bounds.reshape(1, NB)})
    return in_maps


def _finish(outs):
    """Fold per-core [128,17] outputs into (ece, acc)."""
    cum_conf = np.zeros(NB, np.float64)
    cum_acc = np.zeros(NB, np.float64)
    cum_cnt = np.zeros(NB, np.float64)
    for o in outs:
        o = np.asarray(o, np.float64)            # [128, 17]
        stats = o[:, :HALF * 2].reshape(HALF, NB, HALF, 2)  # [g][j][g2][s]
        cnt = o[:, HALF * 2].reshape(HALF, NB)              # [g][j]
        for g in range(HALF):
            cum_conf += stats[g, :, g, 0]
            cum_acc += stats[g, :, g, 1]
        cum_cnt += cnt.sum(axis=0)

    count = cum_cnt[:-1] - cum_cnt[1:]
    sconf = cum_conf[:-1] - cum_conf[1:]
    sacc = cum_acc[:-1] - cum_acc[1:]

    safe = count > 0
    denom = np.where(safe, count, 1.0)
    conf_in = sconf / denom
    acc_in = sacc / denom
    prop = count / float(N)
    ece = float(np.where(safe, np.abs(conf_in - acc_in) * prop, 0.0).sum() * 100.0)
    acc = float(np.where(safe, acc_in * prop, 0.0).sum() * 100.0)
    return (
        np.array([ece], np.float32),
        np.array([acc], np.float32),
    )


def _run(logits, labels, trace=False):
    from concourse.bass_utils import run_bass_kernel_spmd

    nc = _get_nc()
    in_maps = _prep_inputs(logits, labels)
    res = run_bass_kernel_spmd(
        nc, in_maps, core_ids=list(range(NCORES)), trace=trace,
    )
    outs = [r["out"] for r in res.results]
    return _finish(outs), res


def kernel(logits, labels):
    out, _ = _run(logits, labels)
    return out

